# revision 13
# baseline (speedup 1.0000x reference)
"""Trainium2 Bass kernel for nn_AttentionSortNet (sparse_attention).

Computes, per (batch*head) slice:
  sq = bucket-mean(q), sk = bucket-mean(k)          # (64, 64) each
  R  = sq @ sk.T * DIM**-0.5                        # (64, 64)
  r  = (log(relu(R)+eps) + gumbel(u)) / T
  log-domain Sinkhorn row/col normalization (u-v form)
  out = exp(r)

Strategy: shard the 32 bh slices across 8 cores (4 bh each, no
communication). On-core, the 4 bh form two PAIRS (bh0,1) and (bh2,3),
each pair processed on stacked [128, *] tiles (bh b2 on partitions
0-63, b3 on 64-127):

- q/k stream in as 1 MiB HWDGE DMAs (FIFO ring, ~350 GB/s measured);
  the LAST k1 MiB is split 512K/256K/256K so the final fold tail is
  short.
- Within-bucket sums: GpSimd does the level-1 halving add (2048->1024)
  for every 1 MiB chunk; DVE runs the remaining halving ladder down to
  [128, 64] and accumulates per-tile sums. Emission order tracks the
  projected execution timeline so the in-order DVE queue never parks a
  data-starved op in front of ready work (ladders for the q1/k1 tiles
  are interleaved into the pair-A sinkhorn's reciprocal gaps).
- Sinkhorn runs in u-v form: P = diag(u) P0 diag(v). Each half
  iteration is two 64x64 PE matvecs (both bh of the pair, into one
  [128,1] PSUM tile) + ONE DVE reciprocal. Matvecs use float32r
  (single-pass fp32, bf16-class precision; verified 6.6e-3 L2 vs the
  2e-2 gate). 7 iterations (iteration 8 moves the result by 6e-3).
- Materialize: Dg = ident_pair*v (DVE), Vb = ones_blockdiag @ Dg (PE
  row-broadcast), P = (P0*u)*Vb (one fused DVE op), then ONE contiguous
  [128,64] store per pair (out rows are (b,i)-major in DRAM).
- gumbel_u loads as two contiguous [128,64] tiles ((b,i)-major), so no
  tiny-descriptor spray in front of the chunk stream.

Built on bacc.Bacc (not raw Bass): its compile pass splits multi-sem
sync waits, which this walrus requires (one wait per instruction).
"""

import sys

for _p in ("/opt/trn_rl_repo",):
    if _p not in sys.path:
        sys.path.insert(0, _p)

import numpy as np

N_CORES = 8
BH = 32
B_PER = BH // N_CORES          # 4 bh per core
SEQ = 8192
D = 64
BUCKET_SIZE = 128
BUCKETS = SEQ // BUCKET_SIZE   # 64 buckets per bh
EPS = 1e-6
TEMP = 0.7
SINKHORN_ITER = 7
# q/k are reduced to bucket *sums*; fold the two 1/128 mean factors and
# the DIM**-0.5 = 1/8 similarity scale into one constant applied at relu.
R_SCALE = 1.0 / (BUCKET_SIZE * BUCKET_SIZE * 8.0)

CHUNK_F = 2048                 # 1 MiB chunk: [128, 2048] f32, 8 KB/partition

_NC_CACHE = None


def _build():
    import concourse.bacc as bacc
    import concourse.mybir as mybir
    import concourse.tile as tile
    from concourse.masks import make_identity
    from contextlib import ExitStack

    fp32 = mybir.dt.float32
    f32r = mybir.dt.float32r
    bf16 = mybir.dt.bfloat16
    AF = mybir.ActivationFunctionType
    ALU = mybir.AluOpType

    from concourse.hw_specs import get_activation_tables
    from concourse.dve_ops import (
        RECIP_APPROX_FAST_CONSTS,
        RECIPROCAL_APPROX_FAST,
    )
    import bass_rust as _bass_rust

    class _Bacc(bacc.Bacc):
        def insert_act_table_loads(self):
            # Restrict Ln/Exp to the combined natural_log_exp set so the
            # greedy chooser stops reloading ACT tables on every switch.
            has_act = any(
                isinstance(i, mybir.InstActivation)
                for b in self.main_func.blocks
                for i in b.instructions
            )
            if not has_act:
                return
            AF2 = mybir.ActivationFunctionType
            tables = []
            for name, funcs in get_activation_tables(self.m.arch).items():
                if name != "natural_log_exp_and_others":
                    funcs = {f for f in funcs if f not in (AF2.Ln, AF2.Exp)}
                tables.append((name, funcs))
            _bass_rust.insert_act_table_loads(self, tables)

    nc = _Bacc("TRN2", target_bir_lowering=False, debug=False)

    q = nc.dram_tensor("q", [B_PER, SEQ, D], fp32, kind="ExternalInput")
    k = nc.dram_tensor("k", [B_PER, SEQ, D], fp32, kind="ExternalInput")
    gu = nc.dram_tensor("gumbel_u", [B_PER, BUCKETS, BUCKETS], fp32,
                        kind="ExternalInput")
    out = nc.dram_tensor("out", [B_PER, BUCKETS, BUCKETS], fp32,
                         kind="ExternalOutput")

    # (b, s, d) -> (global bucket row, within-bucket payload)
    qv = q.ap().rearrange("b (bk w) d -> (b bk) (w d)", bk=BUCKETS)
    kv = k.ap().rearrange("b (bk w) d -> (b bk) (w d)", bk=BUCKETS)
    guv = gu.ap().rearrange("b i j -> (b i) j")    # [256, 64] rows contiguous
    outv = out.ap().rearrange("b i j -> (b i) j")  # [256, 64]

    H = CHUNK_F // 2

    with tile.TileContext(nc) as tc, ExitStack() as ctx:
        consts = ctx.enter_context(tc.tile_pool(name="consts", bufs=1))
        chunks = ctx.enter_context(tc.tile_pool(name="chunks", bufs=16))
        parts = ctx.enter_context(tc.tile_pool(name="parts", bufs=1))
        sums = ctx.enter_context(tc.tile_pool(name="sums", bufs=1))
        mats = ctx.enter_context(tc.tile_pool(name="mats", bufs=1))
        small = ctx.enter_context(tc.tile_pool(name="small", bufs=1))
        tpsum = ctx.enter_context(tc.tile_pool(name="tpsum", bufs=2, space="PSUM"))
        tqsum = ctx.enter_context(tc.tile_pool(name="tqsum", bufs=2, space="PSUM"))
        rpsum = ctx.enter_context(tc.tile_pool(name="rpsum", bufs=1, space="PSUM"))
        vbpsum = ctx.enter_context(tc.tile_pool(name="vbpsum", bufs=1, space="PSUM"))
        vpsum = ctx.enter_context(tc.tile_pool(name="vpsum", bufs=2, space="PSUM"))

        # ---- phase A: DMA triggers up front, in arrival order. The HWDGE
        # ring drains FIFO so completion order == trigger order.
        CH = {}

        def trig_chunk(tag, c):
            view = qv if tag in ("q0", "q1") else kv
            t = {"q0": 0, "k0": 0, "q1": 1, "k1": 1}[tag]
            ch = chunks.tile([128, CHUNK_F], fp32, tag="chunk",
                             name=f"ch_{tag}{c}")
            nc.sync.dma_start(
                out=ch[:],
                in_=view[128 * t:128 * (t + 1),
                         CHUNK_F * c:CHUNK_F * (c + 1)],
            )
            CH[(tag, c)] = ch

        for c in range(4):
            trig_chunk("q0", c)
        for c in range(3):
            trig_chunk("k0", c)
        # gumbel pair tiles: contiguous 32 KB each, cheap descriptors
        u01 = small.tile([128, BUCKETS], fp32, tag="u01")
        u23 = small.tile([128, BUCKETS], fp32, tag="u23")
        nc.sync.dma_start(out=u01[:], in_=guv[0:128, :])
        nc.sync.dma_start(out=u23[:], in_=guv[128:256, :])
        trig_chunk("k0", 3)
        for c in range(4):
            trig_chunk("q1", c)
        for c in range(3):
            trig_chunk("k1", c)
        # last k1 MiB split 512K/256K/256K into one tile
        k1e = chunks.tile([128, CHUNK_F], fp32, tag="chunk", name="ch_k1e")
        base = CHUNK_F * 3
        nc.sync.dma_start(out=k1e[:, 0:1024], in_=kv[128:256, base:base + 1024])
        nc.sync.dma_start(out=k1e[:, 1024:1536],
                          in_=kv[128:256, base + 1024:base + 1536])
        nc.sync.dma_start(out=k1e[:, 1536:2048],
                          in_=kv[128:256, base + 1536:base + 2048])
        CH[("k1", 3)] = k1e

        # ---- constants on GpSimd (before the first chunk lands)
        ident128 = consts.tile([128, 128], fp32)
        make_identity(nc, ident128[:])
        identp = consts.tile([128, D], fp32)       # stacked I64 pair
        make_identity(nc, identp[0:64, :])
        make_identity(nc, identp[64:128, :])
        ones_bd = consts.tile([128, 128], bf16)    # block-diag ones
        nc.gpsimd.memset(ones_bd[:], 0.0)
        nc.gpsimd.memset(ones_bd[0:64, 0:64], 1.0)
        nc.gpsimd.memset(ones_bd[64:128, 64:128], 1.0)
        epsb = consts.tile([128, 1], fp32)
        nc.gpsimd.memset(epsb[:], EPS)

        # ---- fold helpers ------------------------------------------------
        S = {}

        def gps_l1(tag, c):
            """GpSimd level-1 halving add: [0:1024) += [1024:2048)."""
            ch = CH[(tag, c)]
            nc.gpsimd.tensor_add(ch[:, 0:H], ch[:, 0:H], ch[:, H:2 * H])

        def lad(tag, c, start, base=0):
            """DVE halving ladder on CH[(tag,c)][:, base:base+start) down to
            [128, 64]; create or accumulate the tile sum S[tag]."""
            ch = CH[(tag, c)]
            m = start // 2
            while m > D:
                nc.vector.tensor_add(ch[:, base:base + m],
                                     ch[:, base:base + m],
                                     ch[:, base + m:base + 2 * m])
                m //= 2
            s = S.get(tag)
            if s is None:
                s = parts.tile([128, D], fp32, tag=f"s_{tag}")
                nc.vector.tensor_add(s[:], ch[:, base:base + D],
                                     ch[:, base + D:base + 2 * D])
                S[tag] = s
            else:
                nc.vector.tensor_add(ch[:, base:base + D],
                                     ch[:, base:base + D],
                                     ch[:, base + D:base + 2 * D])
                nc.vector.tensor_add(s[:], s[:], ch[:, base:base + D])

        def sums_T(tag):
            """[128 rows, 64 d] -> [64 d, 128 rows] via PE + ACT copy."""
            tp = tpsum.tile([64, 128], fp32, tag="tp", name=f"tp_{tag}")
            nc.tensor.transpose(tp[:], S[tag][:], ident128[:])
            st = sums.tile([64, 128], f32r, tag=f"T_{tag}")
            nc.scalar.copy(st[:], tp[:])
            return st

        # ---- pair state --------------------------------------------------
        P0fs, P0s, P0Ts, Us, Vs = {}, {}, {}, {}, {}

        def pair_init(p, qT, kT, gup):
            """R matmul + gumbel init for pair p; leaves P0 (fp32 + f32r
            chain-weights copy) and u1 [128,1] f32r."""
            rp = rpsum.tile([128, 128], fp32, tag="rp", name=f"rp{p}")
            nc.tensor.matmul(rp[:], qT[:], kT[:], start=True, stop=True)
            t1 = mats.tile([128, D], fp32, tag=f"t1_{p}")
            nc.scalar.activation(out=t1[0:64, :], in_=rp[0:64, 0:64],
                                 func=AF.Relu, scale=R_SCALE)
            nc.scalar.activation(out=t1[64:128, :], in_=rp[64:128, 64:128],
                                 func=AF.Relu, scale=R_SCALE)
            nc.scalar.activation(out=t1[:], in_=t1[:], func=AF.Ln, bias=epsb[:])
            nc.vector.tensor_sub(t1[:], t1[:], gup[:])
            P0 = mats.tile([128, D], fp32, tag=f"P0_{p}")
            w0 = small.tile([128, 1], fp32, tag=f"w0_{p}")
            nc.scalar.activation(out=P0[:], in_=t1[:], func=AF.Exp,
                                 scale=1.0 / TEMP, accum_out=w0[:])
            u1 = small.tile([128, 1], fp32, tag=f"uvu_{p}", name=f"u1_{p}")
            nc.vector.reciprocal_approx_fast(u1[:], w0[:])
            P0fs[p], P0s[p], Us[p] = P0, P0, u1

        def pair_transpose(p):
            """P0T [128,64]: partitions 0-63 = P0_b2^T, 64-127 = P0_b3^T.
            Two PE transposes (identity partition-matched) + ACT copies."""
            P0 = P0fs[p]
            tpl = tqsum.tile([64, 64], fp32, tag="tq", name=f"tpl{p}")
            nc.tensor.transpose(tpl[:], P0[0:64, :], identp[0:64, :])
            tph = tqsum.tile([64, 64], fp32, tag="tq", name=f"tph{p}")
            nc.tensor.transpose(tph[:], P0[64:128, :], identp[64:128, :])
            P0T = mats.tile([128, D], fp32, tag=f"P0T_{p}")
            nc.scalar.copy(P0T[0:64, :], tpl[:])
            nc.scalar.copy(P0T[64:128, :], tph[:])
            P0Ts[p] = P0T

        def mm_pair(p, W, x, rhs):
            """Two 64x64 fp32r matvecs (both bh) into one [128,1] PSUM."""
            nc.tensor.matmul(x[0:64, :], W[0:64, :], rhs[0:64, :],
                             start=True, stop=True)
            nc.tensor.matmul(x[64:128, :], W[64:128, :], rhs[64:128, :],
                             start=True, stop=True)

        def level_mm(p, kind, t):
            """v-step: x = P0^T u (lhsT=P0); u-step: x = P0 v (lhsT=P0T)."""
            W, rhs = (P0s[p], Us[p]) if kind == "v" else (P0Ts[p], Vs[p])
            x = vpsum.tile([128, 1], fp32, tag="mv", name=f"x{p}{kind}{t}")
            mm_pair(p, W, x, rhs)
            return x

        def level_recip(p, kind, t, x):
            nu = small.tile([128, 1], fp32, tag=f"uv{kind}_{p}",
                            name=f"{kind}{t}_{p}")
            nc.vector.reciprocal_approx_fast(nu[:], x[:])
            if kind == "v":
                Vs[p] = nu
            else:
                Us[p] = nu

        def materialize(p, rows):
            """out rows = (P0 * u) * rowbcast(v); one contiguous store."""
            u_fin = Us[p][:]
            v_fin = Vs[p][:]
            Dg = mats.tile([128, D], bf16, tag=f"Dg_{p}")
            nc.vector.tensor_scalar_mul(Dg[:], identp[:], v_fin)
            vb = vbpsum.tile([128, D], fp32, tag="vb", name=f"vb{p}")
            nc.tensor.matmul(vb[:], ones_bd[:], Dg[:], start=True, stop=True)
            Pm = mats.tile([128, D], fp32, tag=f"P_{p}")
            nc.vector.scalar_tensor_tensor(
                out=Pm[:], in0=P0fs[p][:], scalar=u_fin, in1=vb[:],
                op0=ALU.mult, op1=ALU.mult,
            )
            nc.sync.dma_start(out=outv[rows[0]:rows[1], :], in_=Pm[:])

        # ---- phase B: emission follows the projected execution timeline.
        # GpSimd stream: L1 for every 1 MiB chunk, in arrival order.
        for tag in ("q0", "k0", "q1", "k1"):
            n = 3 if tag == "k1" else 4
            for c in range(n):
                gps_l1(tag, c)

        # DVE / PE / ACT streams in timeline order.
        for c in range(4):
            lad("q0", c, H)
        qT0 = sums_T("q0")
        # gumbel logit prep: g = ln(-ln(u+eps)+eps)  (ACT, early)
        for ut in (u01, u23):
            nc.scalar.activation(out=ut[:], in_=ut[:], func=AF.Ln, bias=epsb[:])
            nc.scalar.activation(out=ut[:], in_=ut[:], func=AF.Ln,
                                 bias=epsb[:], scale=-1.0)
        for c in range(4):
            lad("k0", c, H)
        kT0 = sums_T("k0")

        pair_init(0, qT0, kT0, u01)

        # chain A with q1 ladder fillers in the reciprocal gaps
        steps = [("v", 1)]
        for t in range(2, SINKHORN_ITER + 1):
            steps.append(("u", t))
            steps.append(("v", t))
        fillers = {2: ("q1", 0), 4: ("q1", 1), 6: ("q1", 2), 8: ("q1", 3)}
        for i, (kind, t) in enumerate(steps):
            x = level_mm(0, kind, t)
            if i == 0:
                pair_transpose(0)
            if i in fillers:
                lad(*fillers[i], H)
            level_recip(0, kind, t, x)

        qT1 = sums_T("q1")
        materialize(0, (0, 128))

        # k1 endgame: ladders in data-arrival order, DVE mostly post-chain-A
        lad("k1", 0, H)
        lad("k1", 1, H)
        lad("k1", 3, 1024, base=0)          # 512K piece (raw, no GPS L1)
        lad("k1", 2, H)
        lad("k1", 3, 512, base=1024)        # 256K piece
        lad("k1", 3, 512, base=1536)        # 256K piece
        kT1 = sums_T("k1")

        pair_init(1, qT1, kT1, u23)
        for i, (kind, t) in enumerate(steps):
            x = level_mm(1, kind, t)
            if i == 0:
                pair_transpose(1)
            level_recip(1, kind, t, x)
        materialize(1, (128, 256))

    return nc


def _get_nc():
    global _NC_CACHE
    if _NC_CACHE is None:
        _NC_CACHE = _build()
        if not _NC_CACHE.is_finalized():
            _NC_CACHE.finalize()
    return _NC_CACHE


def _shard(q, k, gumbel_u):
    return [
        {
            "q": np.ascontiguousarray(q[B_PER * c:B_PER * (c + 1)]),
            "k": np.ascontiguousarray(k[B_PER * c:B_PER * (c + 1)]),
            "gumbel_u": np.ascontiguousarray(gumbel_u[B_PER * c:B_PER * (c + 1)]),
        }
        for c in range(N_CORES)
    ]


def kernel(q, k, gumbel_u, **_unused):
    from concourse.bass_utils import run_bass_kernel_spmd

    q = np.asarray(q, dtype=np.float32)
    k = np.asarray(k, dtype=np.float32)
    gumbel_u = np.asarray(gumbel_u, dtype=np.float32)

    nc = _get_nc()
    res = run_bass_kernel_spmd(nc, _shard(q, k, gumbel_u),
                               core_ids=list(range(N_CORES)))
    return np.concatenate([r["out"] for r in res.results], axis=0)
